# revision 5
# baseline (speedup 1.0000x reference)
"""Trainium2 Bass kernel for the ExportableStudentSNN1d problem.

Data-parallel over batch: 64 samples -> 8 cores x 8 samples. Each core runs
an identical NEFF on its batch shard; host concatenates the [8, 4] outputs.

Math notes (TAU1 = 1.0 makes layer-1 LIF memoryless):
  s1_t = (conv1(x_t)*G + b1*G >= TH1)        <=> conv1(x_t) >= TH1/G - b1
  layer2: v2pre = (10/9)*a2 - (1/9)*v2_prev,  a2 = G*(conv2(s1_t) + b2)
     y      = (10/3)*conv2 + carry            (carry = -(1/9)*v2_prev)
     sp     = (y >= TH2 - b2p),  b2p = (10/3)*b2
     carry' = (y + b2p) * ((sp - 1)/9)
  out[b,c] = (sum_{t,l} sp)/(T*L) @ Wfc.T + bfc
"""

import numpy as np

import concourse.bacc as bacc
import concourse.tile as tile
import concourse.mybir as mybir
from concourse.bass_utils import run_bass_kernel_spmd

F32 = mybir.dt.float32
F32R = mybir.dt.float32r

N_CORES = 8
B, C_IN, L, T = 64, 12, 2048, 20
C1, C2, K, PAD = 128, 256, 9, 4
GAIN, TAU2, TH1, TH2 = 3.0, 0.9, 0.02, 0.02
NCLS = 4
B_SH = B // N_CORES            # 8 samples per core
LH = 1024                      # L processed in halves
HALO = 8                       # x halo per side (conv1 then conv2 shifts)
S1W = LH + 2 * PAD             # 1032 s1 columns needed per L-half
XW = LH + 2 * HALO             # 1040 x columns staged per L-half
A2S = (10.0 / 9.0) * GAIN      # 10/3: multiplier on conv2 psum
MDECAY = 1.0 / 9.0

_CACHE = {}


def _build():
    nc = bacc.Bacc("TRN2", target_bir_lowering=False, debug=False)

    x_d = nc.dram_tensor("x", [B_SH, C_IN, L, T], F32R, kind="ExternalInput")
    w1t_d = nc.dram_tensor("w1t", [C_IN, K * C1], F32R, kind="ExternalInput")
    w2t_d = nc.dram_tensor("w2t", [C1, K * C2], F32R, kind="ExternalInput")
    th1_d = nc.dram_tensor("th1", [C1, 1], F32, kind="ExternalInput")
    th2_d = nc.dram_tensor("th2", [C1, 2], F32, kind="ExternalInput")
    b2p_d = nc.dram_tensor("b2p", [C1, 2], F32, kind="ExternalInput")
    wfc_d = nc.dram_tensor("wfc", [C1, 2 * NCLS], F32, kind="ExternalInput")
    bfc_d = nc.dram_tensor("bfc", [NCLS, 1], F32, kind="ExternalInput")
    out_d = nc.dram_tensor("out", [B_SH, NCLS], F32, kind="ExternalOutput")

    with tile.TileContext(nc) as tc:
        with (
            tc.tile_pool(name="const", bufs=1) as cpool,
            tc.tile_pool(name="xstage", bufs=1) as xpool,
            tc.tile_pool(name="xt", bufs=3) as xtpool,
            tc.tile_pool(name="s1", bufs=2) as s1pool,
            tc.tile_pool(name="lif", bufs=3) as lifpool,
            tc.tile_pool(name="carry", bufs=2) as cpool2,
            tc.tile_pool(name="psum1", bufs=1, space="PSUM") as pp1,
            tc.tile_pool(name="psum2", bufs=2, space="PSUM") as pp2,
            tc.tile_pool(name="psfc", bufs=1, space="PSUM") as ppfc,
        ):
            # ---- constants / weights (resident) ----
            w1t = cpool.tile([C_IN, K * C1], F32R)
            nc.sync.dma_start(w1t[:], w1t_d.ap())
            w2t = cpool.tile([C1, K * C2], F32R)
            nc.sync.dma_start(w2t[:], w2t_d.ap())
            th1 = cpool.tile([C1, 1], F32)
            nc.sync.dma_start(th1[:], th1_d.ap())
            th2 = cpool.tile([C1, 2], F32)
            nc.sync.dma_start(th2[:], th2_d.ap())
            b2p = cpool.tile([C1, 2], F32)
            nc.sync.dma_start(b2p[:], b2p_d.ap())
            wfc = cpool.tile([C1, 2 * NCLS], F32)
            nc.sync.dma_start(wfc[:], wfc_d.ap())
            bfc = cpool.tile([NCLS, 1], F32)
            nc.sync.dma_start(bfc[:], bfc_d.ap())
            # spike counts, one column per (h, b, lh, t)
            acc = cpool.tile([C1, 2 * B_SH * 2 * T], F32)

            for b in range(B_SH):
                for lh in range(2):
                    l0 = lh * LH
                    src_lo = max(0, l0 - HALO)
                    src_hi = min(L, l0 + LH + HALO)
                    dst_lo = src_lo - (l0 - HALO)
                    dst_hi = dst_lo + (src_hi - src_lo)

                    xs = xpool.tile([C_IN, XW * T], F32R)
                    xsv = xs[:].rearrange("p (l t) -> p l t", t=T)
                    xsu = xs[:].bitcast(mybir.dt.uint32)
                    if dst_lo > 0:
                        nc.gpsimd.memset(xsu[:, 0 : dst_lo * T], 0)
                    if dst_hi < XW:
                        nc.gpsimd.memset(xsu[:, dst_hi * T : XW * T], 0)
                    nc.sync.dma_start(
                        xsv[:, dst_lo:dst_hi, :], x_d.ap()[b, :, src_lo:src_hi, :]
                    )

                    carry = cpool2.tile([C1, 2 * LH], F32)
                    nc.gpsimd.memset(carry[:], 0.0)

                    for t in range(T):
                        col = b * (2 * T) + lh * T + t

                        # deinterleave timestep t: [12, XW] (f32r, unit stride)
                        xt = xtpool.tile([C_IN, XW], F32R)
                        nc.gpsimd.tensor_copy(xt[:], xsv[:, :, t])

                        # conv1 -> psum1[:, 0:S1W]
                        p1 = pp1.tile([C1, 1536], F32)
                        for c0, cn in ((0, 512), (512, 512), (1024, S1W - 1024)):
                            for k in range(K):
                                nc.tensor.matmul(
                                    p1[:, c0 : c0 + cn],
                                    w1t[:, k * C1 : (k + 1) * C1],
                                    xt[:, c0 + k : c0 + k + cn],
                                    start=(k == 0),
                                    stop=(k == K - 1),
                                )

                        # s1 = (conv1 >= th1_pp), kept in f32r for conv2
                        s1 = s1pool.tile([C1, S1W], F32R)
                        nc.vector.tensor_scalar(
                            s1[:], p1[:, 0:S1W], th1[:], None,
                            op0=mybir.AluOpType.is_ge,
                        )

                        # conv2 per output half + LIF
                        ys = []
                        sps = []
                        ms = []
                        for h in range(2):
                            p2 = pp2.tile([C1, LH], F32)
                            for c0 in (0, 512):
                                for k in range(K):
                                    nc.tensor.matmul(
                                        p2[:, c0 : c0 + 512],
                                        w2t[:, k * C2 + h * C1 : k * C2 + h * C1 + C1],
                                        s1[:, c0 + k : c0 + k + 512],
                                        start=(k == 0),
                                        stop=(k == K - 1),
                                    )

                            ch = carry[:, h * LH : (h + 1) * LH]
                            y = lifpool.tile([C1, LH], F32, tag="y")
                            nc.vector.scalar_tensor_tensor(
                                y[:], p2[:], A2S, ch,
                                op0=mybir.AluOpType.mult, op1=mybir.AluOpType.add,
                            )
                            sp = lifpool.tile([C1, LH], F32, tag="sp")
                            nc.vector.tensor_scalar(
                                sp[:], y[:], th2[:, h : h + 1], 0.0,
                                op0=mybir.AluOpType.is_ge, op1=mybir.AluOpType.add,
                                accum_out=acc[:, col + h * (B_SH * 2 * T) :
                                              col + h * (B_SH * 2 * T) + 1],
                            )
                            m = lifpool.tile([C1, LH], F32, tag="m")
                            nc.scalar.activation(
                                m[:], sp[:], mybir.ActivationFunctionType.Copy,
                                bias=-MDECAY, scale=MDECAY,
                            )
                            ys.append(y)
                            sps.append(sp)
                            ms.append(m)

                        # carry updates last so DVE isn't stalled on ACT
                        for h in range(2):
                            ch = carry[:, h * LH : (h + 1) * LH]
                            nc.vector.scalar_tensor_tensor(
                                ch, ys[h][:], b2p[:, h : h + 1], ms[h][:],
                                op0=mybir.AluOpType.add, op1=mybir.AluOpType.mult,
                            )

            # ---- pooling + FC head ----
            # acc viewed [C1, h, b, lh*t] -> sum innermost 2T*2 cols per (h,b)
            pooled = cpool.tile([C1, 2 * B_SH], F32)
            nc.vector.tensor_reduce(
                pooled[:],
                acc[:].rearrange("p (h b c) -> p (h b) c", h=2, b=B_SH),
                axis=mybir.AxisListType.X, op=mybir.AluOpType.add,
            )
            pfc = ppfc.tile([NCLS, B_SH], F32)
            for h in range(2):
                nc.tensor.matmul(
                    pfc[:],
                    wfc[:, h * NCLS : (h + 1) * NCLS],
                    pooled[:, h * B_SH : (h + 1) * B_SH],
                    start=(h == 0),
                    stop=(h == 1),
                )
            fin = cpool.tile([NCLS, B_SH], F32)
            nc.scalar.activation(
                fin[:], pfc[:], mybir.ActivationFunctionType.Identity,
                bias=bfc[:], scale=1.0 / float(T * L),
            )
            nc.sync.dma_start(out_d.ap().rearrange("b c -> c b"), fin[:])

    nc.compile()
    return nc


def _prep_consts(W1, b1, W2, b2, Wfc, bfc):
    # lhsT layouts: [ci, (k, co)]
    w1t = np.ascontiguousarray(W1.transpose(1, 2, 0)).reshape(C_IN, K * C1)
    w2t = np.ascontiguousarray(W2.transpose(1, 2, 0)).reshape(C1, K * C2)
    th1 = (TH1 / GAIN - b1).reshape(C1, 1).astype(np.float32)
    b2p_full = (A2S * b2).astype(np.float32)          # [256]
    th2_full = (TH2 - b2p_full).astype(np.float32)
    th2 = th2_full.reshape(2, C1).T.copy()            # [128, 2] cols = halves
    b2p = b2p_full.reshape(2, C1).T.copy()
    wfcT = Wfc.T.reshape(2, C1, NCLS)                 # [2, 128, 4]
    wfc_t = wfcT.transpose(1, 0, 2).reshape(C1, 2 * NCLS).copy()
    bfc_c = bfc.reshape(NCLS, 1).astype(np.float32)
    return {
        "w1t": w1t.astype(np.float32),
        "w2t": w2t.astype(np.float32),
        "th1": th1,
        "th2": th2,
        "b2p": b2p,
        "wfc": wfc_t.astype(np.float32),
        "bfc": bfc_c,
    }


def kernel(x, W1, b1, W2, b2, Wfc, bfc, _trace=False):
    x = np.asarray(x, dtype=np.float32)
    consts = _prep_consts(
        np.asarray(W1, np.float32), np.asarray(b1, np.float32),
        np.asarray(W2, np.float32), np.asarray(b2, np.float32),
        np.asarray(Wfc, np.float32), np.asarray(bfc, np.float32),
    )
    if "nc" not in _CACHE:
        _CACHE["nc"] = _build()
    nc = _CACHE["nc"]

    in_maps = []
    for c in range(N_CORES):
        m = dict(consts)
        m["x"] = np.ascontiguousarray(x[c * B_SH : (c + 1) * B_SH])
        in_maps.append(m)

    res = run_bass_kernel_spmd(
        nc, in_maps, core_ids=list(range(N_CORES)), trace=_trace
    )
    out = np.concatenate([res.results[c]["out"] for c in range(N_CORES)], axis=0)
    out = out.astype(np.float32)
    if _trace:
        return out, res
    return out


# revision 40
# speedup vs baseline: 2.1876x; 2.1876x over previous
"""Trainium2 Bass kernel for the ExportableStudentSNN1d problem.

Data-parallel over batch: 64 samples -> 8 cores x 8 samples. Each core runs
an identical NEFF on its batch shard; host concatenates the [8, 4] outputs.

Math notes (TAU1 = 1.0 makes layer-1 LIF memoryless):
  s1_t = (conv1(x_t)*G + b1*G >= TH1)        <=> conv1(x_t) >= TH1/G - b1
  layer2: v2pre = (10/9)*a2 - (1/9)*v2_prev,  a2 = G*(conv2(s1_t) + b2)
     psum2  = conv2 with W2 pre-scaled by 10/3 on host
     y      = (psum2 + b2p) + carry = v2pre   (b2p = (10/3)b2, carry = -(1/9)v2)
     m'     = (y < TH2) * (-1/9)
     carry' = y * m'
     spike counts via ACT Sign(y - TH2) accumulation: count = (sum_sign + n)/2,
     folded into the FC scale/bias on host.
  out[b,c] = (sum_{t,l} sp)/(T*L) @ Wfc.T + bfc

Layout: host pre-transposes x to [B, Cin, T, L] and casts to bf16 so the
conv rhs reads are unit-stride; both convs run in bf16 (fp32 PSUM accum).
conv1's im2col is materialized by DMA: 9 tap-shifted replicas of x across
108 SBUF partitions, so conv1 is a single K=108 matmul per 512-column
chunk (vs 9 accumulating K=12 matmuls).
"""

import numpy as np
import ml_dtypes

import concourse.bacc as bacc
import concourse.tile as tile
import concourse.mybir as mybir
from concourse.bass_utils import run_bass_kernel_spmd

F32 = mybir.dt.float32
BF16 = mybir.dt.bfloat16

N_CORES = 8
B, C_IN, L, T = 64, 12, 2048, 20
C1, C2, K, PAD = 128, 256, 9, 4
GAIN, TAU2, TH1, TH2 = 3.0, 0.9, 0.02, 0.02
NCLS = 4
B_SH = B // N_CORES            # 8 samples per core
LH = 1024                      # L processed in halves
HALO = 8                       # x halo per side (conv1 then conv2 shifts)
S1W = LH + 2 * PAD             # 1032 s1 columns needed per L-half
XW = LH + 2 * HALO             # 1040 x columns staged per L-half
A2S = (10.0 / 9.0) * GAIN      # 10/3: multiplier on conv2 psum
MDECAY = 1.0 / 9.0

_CACHE = {}


def _build():
    nc = bacc.Bacc("TRN2", target_bir_lowering=False, debug=False)

    x_d = nc.dram_tensor("x", [B_SH, C_IN, T, L], BF16, kind="ExternalInput")
    w1t_d = nc.dram_tensor("w1t", [K * C_IN, C1], BF16, kind="ExternalInput")
    w2t_d = nc.dram_tensor("w2t", [C1, K * C2], BF16, kind="ExternalInput")
    th1_d = nc.dram_tensor("th1", [C1, 1], F32, kind="ExternalInput")
    b2p_d = nc.dram_tensor("b2p", [C1, 2], F32, kind="ExternalInput")
    wfc_d = nc.dram_tensor("wfc", [C1, 2 * NCLS], F32, kind="ExternalInput")
    bfc_d = nc.dram_tensor("bfc", [NCLS, 1], F32, kind="ExternalInput")
    out_d = nc.dram_tensor("out", [B_SH, NCLS], F32, kind="ExternalOutput")

    with tile.TileContext(nc) as tc:
        with (
            tc.tile_pool(name="const", bufs=1) as cpool,
            tc.tile_pool(name="xstage", bufs=2) as xpool,
            tc.tile_pool(name="s1", bufs=2) as s1pool,
            tc.tile_pool(name="lif", bufs=3) as lifpool,
            tc.tile_pool(name="carry", bufs=2) as cpool2,
            tc.tile_pool(name="psum1", bufs=1, space="PSUM") as pp1,
            tc.tile_pool(name="psum2", bufs=2, space="PSUM") as pp2,
            tc.tile_pool(name="psfc", bufs=1, space="PSUM") as ppfc,
        ):
            # ---- constants / weights (resident) ----
            # w1t rows (12k+ci) hold W1[:, ci, k] (im2col layout)
            w1t = cpool.tile([K * C_IN, C1], BF16)
            nc.sync.dma_start(w1t[:], w1t_d.ap())
            w2t = cpool.tile([C1, K * C2], BF16)
            nc.sync.dma_start(w2t[:], w2t_d.ap())
            th1 = cpool.tile([C1, 1], F32)
            nc.sync.dma_start(th1[:], th1_d.ap())
            b2p = cpool.tile([C1, 2], F32)
            nc.sync.dma_start(b2p[:], b2p_d.ap())
            nth2 = cpool.tile([C1, 1], F32)
            nc.gpsimd.memset(nth2[:], -TH2)
            wfc = cpool.tile([C1, 2 * NCLS], F32)
            nc.sync.dma_start(wfc[:], wfc_d.ap())
            bfc = cpool.tile([NCLS, 1], F32)
            nc.sync.dma_start(bfc[:], bfc_d.ap())
            # spike counts, one column per (h, b, lh, t)
            acc = cpool.tile([C1, 2 * B_SH * 2 * T], F32)

            segs = [(b, lh) for b in range(B_SH) for lh in range(2)]

            def stage_segment(idx):
                # im2col staging: rows (12k+ci) = x[ci] shifted by tap k.
                # column (t, c) of row-group k = x[b, ci, t, l0+c+k-8]
                b, lh = segs[idx]
                l0 = lh * LH
                xs = xpool.tile([K * C_IN, T * S1W], BF16)
                xsv = xs[:].rearrange("p (t c) -> p t c", c=S1W)
                # zero the possible halo bands (32-aligned base partition
                # required for engine ops -> memset all rows; the DMAs
                # below overwrite whatever is valid)
                if l0 == 0:
                    nc.gpsimd.memset(xsv[:, :, 0:HALO], 0.0)
                if l0 + LH == L:
                    nc.gpsimd.memset(xsv[:, :, S1W - HALO : S1W], 0.0)
                for k in range(K):
                    rows = slice(C_IN * k, C_IN * (k + 1))
                    c_lo = max(0, HALO - k - l0)
                    c_hi = min(S1W, L - l0 - k + HALO)
                    src = x_d.ap()[b, :, :,
                                   l0 + c_lo + k - HALO : l0 + c_hi + k - HALO]
                    if idx == 0:
                        # cold start: split so the first timesteps' columns
                        # land first
                        nc.sync.dma_start(
                            xsv[rows, 0:2, c_lo:c_hi], src[:, 0:2, :])
                        nc.sync.dma_start(
                            xsv[rows, 2:T, c_lo:c_hi], src[:, 2:T, :])
                    else:
                        nc.sync.dma_start(xsv[rows, :, c_lo:c_hi], src)
                carry = cpool2.tile([C1, 2 * LH], F32)
                nc.gpsimd.memset(carry[:], 0.0)
                return xs, carry

            def conv1_block(xs, t):
                # conv1: one K=108 matmul per chunk, then s1 on DVE
                p1 = pp1.tile([C1, 1536], F32)
                for c0, cn in ((0, 512), (512, 512), (1024, S1W - 1024)):
                    nc.tensor.matmul(
                        p1[:, c0 : c0 + cn],
                        w1t[:],
                        xs[:, t * S1W + c0 : t * S1W + c0 + cn],
                        start=True,
                        stop=True,
                    )
                s1 = s1pool.tile([C1, S1W], BF16)
                nc.vector.tensor_scalar(
                    s1[:], p1[:, 0:S1W], th1[:], None,
                    op0=mybir.AluOpType.is_ge,
                )
                return s1

            def conv2_block(s1, h):
                p2 = pp2.tile([C1, LH], F32)
                for c0 in (0, 512):
                    for k in range(K):
                        nc.tensor.matmul(
                            p2[:, c0 : c0 + 512],
                            w2t[:, k * C2 + h * C1 : k * C2 + h * C1 + C1],
                            s1[:, c0 + k : c0 + k + 512],
                            start=(k == 0),
                            stop=(k == K - 1),
                        )
                return p2

            def lif_front(carry, p2, h):
                # y = (psum2 + b2p) + carry;  m' = (y < TH2)*(-1/9)
                ch = carry[:, h * LH : (h + 1) * LH]
                y = lifpool.tile([C1, LH], F32, tag="y")
                nc.vector.scalar_tensor_tensor(
                    y[:], p2[:], b2p[:, h : h + 1], ch,
                    op0=mybir.AluOpType.add, op1=mybir.AluOpType.add,
                )
                m = lifpool.tile([C1, LH], F32, tag="m")
                nc.vector.tensor_scalar(
                    m[:], y[:], TH2, -MDECAY,
                    op0=mybir.AluOpType.is_lt, op1=mybir.AluOpType.mult,
                )
                return y, m

            def lif_back(carry, y, m, h, col):
                # sign-sum for spike counting (off the carry chain)
                sg = lifpool.tile([C1, LH], F32, tag="sg")
                nc.scalar.activation(
                    sg[:], y[:], mybir.ActivationFunctionType.Sign,
                    bias=nth2[:],
                    accum_out=acc[:, col + h * (B_SH * 2 * T) :
                                  col + h * (B_SH * 2 * T) + 1],
                )
                # carry chain tail on gpsimd
                ch = carry[:, h * LH : (h + 1) * LH]
                nc.gpsimd.tensor_tensor(
                    ch, y[:], m[:], op=mybir.AluOpType.mult,
                )

            # conv1 of segment idx+1's t=0 fills the empty t=19 pipeline
            # slot of segment idx, so segment boundaries don't stall PE
            staged = stage_segment(0)
            s1_cur = conv1_block(staged[0], 0)
            for idx in range(len(segs)):
                b, lh = segs[idx]
                xs, carry = staged
                if idx + 1 < len(segs):
                    staged = stage_segment(idx + 1)
                for t in range(T):
                    col = b * (2 * T) + lh * T + t
                    p2_0 = conv2_block(s1_cur, 0)
                    # emit conv1(t+1)+s1(t+1) before the h0 LIF ops: PE order
                    # is unchanged, but s1 lands ~2us earlier on DVE so
                    # conv2(t+1,h0) never waits on it
                    if t + 1 < T:
                        s1_next = conv1_block(xs, t + 1)
                    elif idx + 1 < len(segs):
                        s1_next = conv1_block(staged[0], 0)
                    else:
                        s1_next = None
                    y0, m0 = lif_front(carry, p2_0, 0)
                    lif_back(carry, y0, m0, 0, col)
                    p2_1 = conv2_block(s1_cur, 1)
                    y1, m1 = lif_front(carry, p2_1, 1)
                    lif_back(carry, y1, m1, 1, col)
                    s1_cur = s1_next

            # ---- pooling + FC head ----
            pooled = cpool.tile([C1, 2 * B_SH], F32)
            nc.vector.tensor_reduce(
                pooled[:],
                acc[:].rearrange("p (h b c) -> p (h b) c", h=2, b=B_SH),
                axis=mybir.AxisListType.X, op=mybir.AluOpType.add,
            )
            pfc = ppfc.tile([NCLS, B_SH], F32)
            for h in range(2):
                nc.tensor.matmul(
                    pfc[:],
                    wfc[:, h * NCLS : (h + 1) * NCLS],
                    pooled[:, h * B_SH : (h + 1) * B_SH],
                    start=(h == 0),
                    stop=(h == 1),
                )
            # pfc holds Wfc @ sign_sums; counts = (sign_sum + T*L)/2 is folded
            # into scale and the host-adjusted bias
            fin = cpool.tile([NCLS, B_SH], F32)
            nc.scalar.activation(
                fin[:], pfc[:], mybir.ActivationFunctionType.Identity,
                bias=bfc[:], scale=1.0 / float(2 * T * L),
            )
            nc.sync.dma_start(out_d.ap().rearrange("b c -> c b"), fin[:])

    nc.compile()
    return nc


def _prep_consts(W1, b1, W2, b2, Wfc, bfc):
    # w1t im2col layout: row (12k+ci), col co = W1[co, ci, k]
    w1t = np.ascontiguousarray(W1.transpose(2, 1, 0)).reshape(K * C_IN, C1)
    # W2 pre-scaled so psum2 = (10/9)*GAIN*conv2 directly
    w2t = np.ascontiguousarray((W2 * A2S).transpose(1, 2, 0)).reshape(C1, K * C2)
    th1 = (TH1 / GAIN - b1).reshape(C1, 1).astype(np.float32)
    b2p_full = (A2S * b2).astype(np.float32)
    b2p = b2p_full.reshape(2, C1).T.copy()            # [128, 2] cols = halves
    wfcT = Wfc.T.reshape(2, C1, NCLS)                 # [2, 128, 4]
    wfc_t = wfcT.transpose(1, 0, 2).reshape(C1, 2 * NCLS).copy()
    # counts = (sign_sum + T*L)/2 folded into the FC epilogue:
    # out = (Wfc @ sign_sum)/(2*T*L) + (bfc + 0.5*rowsum(Wfc))
    bfc_c = (bfc + 0.5 * Wfc.sum(axis=1)).reshape(NCLS, 1).astype(np.float32)
    return {
        "w1t": w1t.astype(ml_dtypes.bfloat16),
        "w2t": w2t.astype(ml_dtypes.bfloat16),
        "th1": th1,
        "b2p": b2p,
        "wfc": wfc_t.astype(np.float32),
        "bfc": bfc_c,
    }


def kernel(x, W1, b1, W2, b2, Wfc, bfc, _trace=False):
    x = np.asarray(x, dtype=np.float32)
    # [B, Cin, L, T] -> [B, Cin, T, L] bf16 so on-chip reads are unit-stride
    x_t = np.ascontiguousarray(x.transpose(0, 1, 3, 2)).astype(ml_dtypes.bfloat16)
    consts = _prep_consts(
        np.asarray(W1, np.float32), np.asarray(b1, np.float32),
        np.asarray(W2, np.float32), np.asarray(b2, np.float32),
        np.asarray(Wfc, np.float32), np.asarray(bfc, np.float32),
    )
    if "nc" not in _CACHE:
        _CACHE["nc"] = _build()
    nc = _CACHE["nc"]

    in_maps = []
    for c in range(N_CORES):
        m = dict(consts)
        m["x"] = np.ascontiguousarray(x_t[c * B_SH : (c + 1) * B_SH])
        in_maps.append(m)

    res = run_bass_kernel_spmd(
        nc, in_maps, core_ids=list(range(N_CORES)), trace=_trace
    )
    out = np.concatenate([res.results[c]["out"] for c in range(N_CORES)], axis=0)
    out = out.astype(np.float32)
    if _trace:
        return out, res
    return out
